# revision 12
# baseline (speedup 1.0000x reference)
"""Trainium2 Bass kernel: European payer swaption MC pricer (Trolle-Schwartz).

Contract: kernel(**inputs) takes FULL unsharded inputs (N=131072 paths),
returns FULL per-path payoff vector [N] float32. Internally shards the
path axis over 8 NeuronCores (embarrassingly parallel MC), runs a Bass
kernel per core via run_bass_kernel_spmd, and re-assembles.

Math notes
----------
Reference simulates, per path, 100 Euler steps of
    vp = max(v,0); sv = sqrt(vp)
    x'  = c*x + sv*dW1               c = 1-g*dt
    v'  = v + kappa*(theta-vp)*dt + sigma*sv*dW2
    p1' = c*p1 + dt*x
    p2' = c*p2 + dt*vp
    p3' = d*p3 + dt*vp               d = 1-2*g*dt
    p4' = c*p4 + dt*p2
    p5' = d*p5 + dt*p3
    p6' = d*p6 + 2*dt*p5
    disc += r(x,p1..p6)*dt           r linear in states
then a payoff from 21 zero-coupon bonds (tau=0 bond is identically 1).

Device-side state is exponentially rescaled (xt = x*c^-t, qi = pi*scale^-t)
so each linear recurrence becomes ONE fused scalar_tensor_tensor op with a
per-step immediate; the per-step scale on dW1 is folded into the host-made
random increments. disc needs the time-sums of all 7 linear states; these
are recovered in closed form from the final states plus just two running
sums (Svp = sum vp_t, Sw1 = sum sv_t*dW1_t) via telescoping:
    S_x  = (x_0  - x_T  + Sw1)    / (1-c)
    S_p1 = (p1_0 - p1_T + dt*S_x) / (1-c)     ... etc.
"""

import numpy as np

N_TOTAL = 131072
N_CORES = 8
PC = N_TOTAL // N_CORES  # paths per core = 16384
P = 128                  # partitions
F = PC // P              # free elems per partition = 128
N_STEPS = 100
ZBLK = 10                # steps per z-stream DMA block

STRIKE = 0.07
EXERCISE = 1.0
DELTA = 0.25
FIRST_FIX = 1.0
LAST_FIX = 5.75
NOTIONAL = 1.0e4
SEED = 1234
N_PAY = int(round((LAST_FIX - FIRST_FIX) / DELTA)) + 1  # 20

_prog_cache = {}


# ---------------------------------------------------------------- host math

def _zcb_coeffs(a, b, g, varphi):
    """B-coefficients of log P(tau) for tau = DELTA*k, k=1..N_PAY."""
    c1 = a / g + b / (g * g)
    taus = DELTA * np.arange(1, N_PAY + 1, dtype=np.float64)
    e1 = np.exp(-g * taus)
    e2 = np.exp(-2.0 * g * taus)
    I0 = (1.0 - e1) / g
    I1 = (1.0 - e1 * (1.0 + g * taus)) / (g * g)
    J0 = (1.0 - e2) / (2.0 * g)
    J1 = (1.0 - e2 * (1.0 + 2.0 * g * taus)) / (4.0 * g * g)
    J2 = (2.0 - e2 * (2.0 + 4.0 * g * taus + 4.0 * (g * taus) ** 2)) / (8.0 * g**3)
    Bx = -(a * I0 + b * I1)
    Bp1 = -b * I0
    Bp2 = -c1 * (a * I0 + b * I1)
    Bp4 = -c1 * b * I0
    Bp3 = a * c1 * J0 + (b * c1 + a * b / g) * J1 + (b * b / g) * J2
    Bp5 = (b * c1 + a * b / g) * J0 + 2.0 * (b * b / g) * J1
    Bp6 = (b * b / g) * J0
    const = -varphi * taus
    return const, Bx, Bp1, Bp2, Bp3, Bp4, Bp5, Bp6


class _Lin(dict):
    """Tiny linear-expression helper: {tile_name: coeff} + 'const'."""

    def __add__(self, o):
        r = _Lin(self)
        for k, v in o.items():
            r[k] = r.get(k, 0.0) + v
        return r

    def __sub__(self, o):
        r = _Lin(self)
        for k, v in o.items():
            r[k] = r.get(k, 0.0) - v
        return r

    def __mul__(self, s):
        return _Lin({k: v * s for k, v in self.items()})

    __rmul__ = __mul__


def _disc_lambda(kappa, theta, sigma, rho, a, b, g, varphi, dt, c, d):
    """Coefficients of -disc as a linear expr over final device tiles."""
    c1 = a / g + b / (g * g)
    c100 = c**N_STEPS
    d100 = d**N_STEPS
    L = lambda **kw: _Lin(kw)
    xT = L(xt=c100)
    p1T = L(q1=c100)
    p2T = L(q2=c100)
    p3T = L(q3=d100)
    p4T = L(q4=c100)
    p5T = L(q5=d100)
    p6T = L(q6=d100)
    Sx = (L(x0=1.0) - xT + L(sw1=1.0)) * (1.0 / (1.0 - c))
    Sp1 = (L(p10=1.0) - p1T + dt * Sx) * (1.0 / (1.0 - c))
    Sp2 = (L(p20=1.0) - p2T + dt * L(svp=1.0)) * (1.0 / (1.0 - c))
    Sp3 = (L(p30=1.0) - p3T + dt * L(svp=1.0)) * (1.0 / (1.0 - d))
    Sp4 = (L(p40=1.0) - p4T + dt * Sp2) * (1.0 / (1.0 - c))
    Sp5 = (L(p50=1.0) - p5T + dt * Sp3) * (1.0 / (1.0 - d))
    Sp6 = (L(p60=1.0) - p6T + 2.0 * dt * Sp5) * (1.0 / (1.0 - d))
    disc = dt * (
        _Lin(const=100.0 * varphi)
        + a * Sx
        + b * Sp1
        + a * c1 * Sp2
        + b * c1 * Sp4
        - a * c1 * Sp3
        - (b * c1 + a * b / g) * Sp5
        - (b * b / g) * Sp6
    )
    return -1.0 * disc


# ---------------------------------------------------------------- program

def _build_program(kappa, theta, sigma, rho, a, b, g, varphi, dt, debug=False):
    import concourse.bass as bass
    import concourse.tile as tile
    from concourse import bacc, mybir

    f32 = mybir.dt.float32
    AL = mybir.AluOpType
    AF = mybir.ActivationFunctionType

    c = 1.0 - g * dt
    d = 1.0 - 2.0 * g * dt
    kdt = kappa * dt
    ktdt = kappa * theta * dt
    c100 = c**N_STEPS
    d100 = d**N_STEPS

    nc = bacc.Bacc("TRN2", target_bir_lowering=False, debug=False,
                   num_devices=N_CORES)

    ins = {}
    for name in ["x0", "v0", "p10", "p20", "p30", "p40", "p50", "p60"]:
        ins[name] = nc.declare_dram_parameter(name, [P, F], f32, isOutput=False)
    dw = nc.declare_dram_parameter("dw", [N_STEPS, P, F, 2], f32, isOutput=False)
    out = nc.declare_dram_parameter("pay", [P, F], f32, isOutput=True)
    dbg_names = ["xt", "q1", "q2", "q3", "q4", "q5", "q6", "sw1", "svp",
                 "nd", "sw", "vt"]
    dbg_out = {}
    if debug:
        for n_ in dbg_names:
            dbg_out[n_] = nc.declare_dram_parameter(
                f"dbg_{n_}", [P, F], f32, isOutput=True)

    with tile.TileContext(nc) as tc:
        with (
            tc.tile_pool(name="state", bufs=1) as st,
            tc.tile_pool(name="z", bufs=3) as zp,
            tc.tile_pool(name="tmp", bufs=3) as tp,
            tc.tile_pool(name="pay", bufs=4) as pp,
        ):
            # persistent state tiles
            xt = st.tile([P, F], f32, tag="xt")
            vt = st.tile([P, F], f32, tag="vt")
            q = {i: st.tile([P, F], f32, tag=f"q{i}", name=f"q{i}")
                 for i in range(1, 7)}
            svp = st.tile([P, F], f32, tag="svp")
            sw1 = st.tile([P, F], f32, tag="sw1")
            # read-only copies of initial states (for disc identities)
            init = {n: st.tile([P, F], f32, tag=f"i_{n}", name=f"i_{n}")
                    for n in ["x0", "p10", "p20", "p30", "p40", "p50", "p60"]}

            nc.sync.dma_start(xt[:], ins["x0"].ap())
            nc.sync.dma_start(vt[:], ins["v0"].ap())
            for i in range(1, 7):
                nc.sync.dma_start(q[i][:], ins[f"p{i}0"].ap())
            for n, t_ in init.items():
                nc.sync.dma_start(t_[:], ins[n].ap())
            nc.vector.memset(svp[:], 0.0)
            nc.vector.memset(sw1[:], 0.0)

            # per-step immediates (host f64 -> f32 imm at emit)
            cinv = [float(c ** (-(t + 1))) for t in range(N_STEPS)]
            dinv = [float(d ** (-(t + 1))) for t in range(N_STEPS)]
            cpw = [float(c ** (t + 1)) for t in range(N_STEPS)]
            s1 = float(dt / c)
            s4 = float(dt / c)
            s5 = float(dt / d)
            s6 = float(2.0 * dt / d)

            ztile = None
            for t in range(N_STEPS):
                if t % ZBLK == 0:
                    ztile = zp.tile([P, ZBLK, F, 2], f32, tag="z")
                    src = dw.ap()[t:t + ZBLK].rearrange("b p f w -> p b f w")
                    nc.sync.dma_start(ztile[:], src)
                j = t % ZBLK
                e1 = ztile[:, j, :, 0]
                e2 = ztile[:, j, :, 1]

                vp = tp.tile([P, F], f32, tag="vp")
                sv = tp.tile([P, F], f32, tag="sv")
                w1c = tp.tile([P, F], f32, tag="w1c")
                w2 = tp.tile([P, F], f32, tag="w2")
                u = tp.tile([P, F], f32, tag="u")

                nc.scalar.activation(vp[:], vt[:], AF.Relu)
                nc.scalar.activation(sv[:], vp[:], AF.Sqrt)
                # plain tensor_tensor ops go to Pool (gpsimd); fused
                # scalar_tensor_tensor is DVE-only on NC v3.
                nc.gpsimd.tensor_tensor(w1c[:], sv[:], e1, AL.mult)
                nc.gpsimd.tensor_tensor(w2[:], sv[:], e2, AL.mult)
                # q6 += s6*q5 ; q4 += s4*q2 ; q5 += s5*q3   (old values)
                nc.vector.scalar_tensor_tensor(
                    q[6][:], q[5][:], s6, q[6][:], AL.mult, AL.add)
                nc.vector.scalar_tensor_tensor(
                    q[4][:], q[2][:], s4, q[4][:], AL.mult, AL.add)
                nc.vector.scalar_tensor_tensor(
                    q[5][:], q[3][:], s5, q[5][:], AL.mult, AL.add)
                # q3 += dt*d^-(t+1)*vp ; q2 += dt*c^-(t+1)*vp
                nc.vector.scalar_tensor_tensor(
                    q[3][:], vp[:], float(dt * dinv[t]), q[3][:], AL.mult, AL.add)
                nc.vector.scalar_tensor_tensor(
                    q[2][:], vp[:], float(dt * cinv[t]), q[2][:], AL.mult, AL.add)
                # q1 += (dt/c)*xt (old) ; then xt += w1c
                nc.vector.scalar_tensor_tensor(
                    q[1][:], xt[:], s1, q[1][:], AL.mult, AL.add)
                nc.gpsimd.tensor_tensor(xt[:], w1c[:], xt[:], AL.add)
                # Sw1 += c^(t+1)*w1c ; Svp += vp
                nc.vector.scalar_tensor_tensor(
                    sw1[:], w1c[:], cpw[t], sw1[:], AL.mult, AL.add)
                nc.gpsimd.tensor_tensor(svp[:], svp[:], vp[:], AL.add)
                # v update: u = -kdt*vp + w2 ; v = (u + ktdt) + v
                nc.vector.scalar_tensor_tensor(
                    u[:], vp[:], float(-kdt), w2[:], AL.mult, AL.add)
                nc.vector.scalar_tensor_tensor(
                    vt[:], u[:], float(ktdt), vt[:], AL.add, AL.add)

            # ---------------- payoff ----------------
            # nd = -disc as linear combo over tiles
            lam = _disc_lambda(kappa, theta, sigma, rho, a, b, g, varphi, dt, c, d)
            tile_of = {"xt": xt, "sw1": sw1, "svp": svp,
                       **{f"q{i}": q[i] for i in range(1, 7)},
                       "x0": init["x0"],
                       **{f"p{i}0": init[f"p{i}0"] for i in range(1, 7)}}
            nd = pp.tile([P, F], f32, tag="nd")
            items = [(k, v) for k, v in lam.items() if k != "const"]
            k0, v0 = items[0]
            nc.vector.tensor_scalar(
                nd[:], tile_of[k0][:], float(v0), float(lam.get("const", 0.0)),
                AL.mult, AL.add)
            for kn, vn in items[1:]:
                nc.vector.scalar_tensor_tensor(
                    nd[:], tile_of[kn][:], float(vn), nd[:], AL.mult, AL.add)
            ed = pp.tile([P, F], f32, tag="ed")
            nc.scalar.activation(ed[:], nd[:], AF.Exp)

            # ZCB exponentials and swap accumulation
            zc, zBx, zB1, zB2, zB3, zB4, zB5, zB6 = _zcb_coeffs(a, b, g, varphi)
            # fold state rescaling into coeffs
            zBx, zB1, zB2, zB4 = zBx * c100, zB1 * c100, zB2 * c100, zB4 * c100
            zB3, zB5, zB6 = zB3 * d100, zB5 * d100, zB6 * d100
            SD = STRIKE * DELTA
            sw = pp.tile([P, F], f32, tag="sw")
            for k in range(N_PAY):  # tau index k+1
                uk = pp.tile([P, F], f32, tag="uk")
                pk = pp.tile([P, F], f32, tag="pk")
                nc.vector.tensor_scalar(
                    uk[:], xt[:], float(zBx[k]), float(zc[k]), AL.mult, AL.add)
                for i, B in enumerate([zB1, zB2, zB3, zB4, zB5, zB6]):
                    nc.vector.scalar_tensor_tensor(
                        uk[:], q[i + 1][:], float(B[k]), uk[:], AL.mult, AL.add)
                nc.scalar.activation(pk[:], uk[:], AF.Exp)
                # swap*NOTIONAL = N*(1 - P20 - SD*sum_k Pk)
                wk = -NOTIONAL * SD if k < N_PAY - 1 else -NOTIONAL * (1.0 + SD)
                if k == 0:
                    nc.vector.tensor_scalar(
                        sw[:], pk[:], float(wk), float(NOTIONAL), AL.mult, AL.add)
                else:
                    nc.vector.scalar_tensor_tensor(
                        sw[:], pk[:], float(wk), sw[:], AL.mult, AL.add)

            if debug:
                dbg_tiles = {"xt": xt, "sw1": sw1, "svp": svp, "nd": nd,
                             "sw": sw, "vt": vt,
                             **{f"q{i}": q[i] for i in range(1, 7)}}
                for n_, t_ in dbg_tiles.items():
                    nc.sync.dma_start(dbg_out[n_].ap(), t_[:])

            pay = pp.tile([P, F], f32, tag="payt")
            nc.vector.tensor_scalar_max(sw[:], sw[:], 0.0)
            nc.vector.tensor_tensor(pay[:], sw[:], ed[:], AL.mult)
            nc.sync.dma_start(out.ap(), pay[:])

    nc.compile()
    return nc


def _get_program(key_vals, debug=False):
    key = tuple(np.float64(v) for v in key_vals) + (debug,)
    if key not in _prog_cache:
        _prog_cache[key] = _build_program(*key_vals, debug=debug)
    return _prog_cache[key]


# ---------------------------------------------------------------- kernel

def kernel(x, v, phi1, phi2, phi3, phi4, phi5, phi6, const, t0, N):
    import jax
    import jax.numpy as jnp
    from concourse.bass_utils import run_bass_kernel_spmd

    n = int(N)
    assert n == N_TOTAL, f"kernel hardcoded for N={N_TOTAL}, got {n}"
    constf = np.asarray(const, np.float64)
    kappa, theta, sigma, rho, a, b, g, varphi = [float(z) for z in constf]
    dt = float((EXERCISE - float(np.asarray(t0))) / N_STEPS)
    c = 1.0 - g * dt
    d = 1.0 - 2.0 * g * dt
    sqdt = np.sqrt(dt)
    srho = np.sqrt(1.0 - rho * rho)

    # --- reproduce the reference's normals exactly ---
    # Same eager call as reference._simulate, NO device/backend pinning:
    # the default PRNG impl here (rbg) is backend-dependent, so matching
    # the reference requires drawing on the same backend the harness's
    # reference run uses — i.e. whatever this environment defaults to.
    zh = np.asarray(jax.random.normal(jax.random.key(SEED), (N_STEPS, n // 2, 2),
                                      dtype=jnp.float32))
    z0 = np.concatenate([zh[:, :, 0], -zh[:, :, 0]], axis=1)  # [T, N]
    z1 = np.concatenate([zh[:, :, 1], -zh[:, :, 1]], axis=1)

    cinv = (c ** -(np.arange(1, N_STEPS + 1, dtype=np.float64))).astype(np.float32)
    e1 = (z0 * np.float32(sqdt)) * cinv[:, None]
    e2 = np.float32(sigma * sqdt) * (np.float32(rho) * z0 + np.float32(srho) * z1)
    dwfull = np.stack([e1, e2], axis=-1)  # [T, N, 2] f32

    nc = _get_program((kappa, theta, sigma, rho, a, b, g, varphi, dt))

    state_in = {"x0": x, "v0": v, "p10": phi1, "p20": phi2, "p30": phi3,
                "p40": phi4, "p50": phi5, "p60": phi6}
    in_maps = []
    for k in range(N_CORES):
        sl = slice(k * PC, (k + 1) * PC)
        m = {name: np.ascontiguousarray(
                np.asarray(arr, np.float32)[sl].reshape(P, F))
             for name, arr in state_in.items()}
        m["dw"] = np.ascontiguousarray(dwfull[:, sl, :].reshape(N_STEPS, P, F, 2))
        in_maps.append(m)

    res = run_bass_kernel_spmd(nc, in_maps, list(range(N_CORES)))
    global _last_result
    _last_result = res
    outs = [res.results[k]["pay"].reshape(PC) for k in range(N_CORES)]
    return np.concatenate(outs).astype(np.float32)


_last_result = None
